# revision 10
# baseline (speedup 1.0000x reference)
"""Trainium2 Bass kernel for nn_MPCActor: MLP (256->512->512->32, relu/relu/
sigmoid) followed by 100 SGD steps on u (closed form, since the per-element
recurrence u <- a*u + b with a = 1-2*lr*q, b = -lr*p has the exact solution
u_N = a^N u0 - 0.5*(p/q)*(1 - a^N)).

Data parallel over 8 NeuronCores: batch 32768 -> 4096 rows per core, MLP
weights replicated. Activations are kept feature-on-partition / batch-on-free
so weights serve as the stationary matmul operand in their natural [in, out]
layout; obs tiles are transposed on the PE. Matmuls run in bf16 (fp32
accumulate in PSUM); everything after the sigmoid stays fp32.

Only the 8 W3 columns that the u-update actually reads (q_u = cols 12:16,
p_u = cols 28:32) are computed; x_init never enters the gradient.
"""

import numpy as np

import concourse.bass as bass
import concourse.mybir as mybir
import concourse.tile as tile
from concourse import bacc, masks
from concourse.bass_utils import run_bass_kernel_spmd

NCORES = 8
BATCH = 32768
BPC = BATCH // NCORES  # 4096 rows per core
OBS = 256
HID = 512
NQP = 8  # q_u (4) + p_u (4) columns of W3 that matter
BT = 512  # batch tile (matmul moving free dim)
NT = BPC // BT  # 8 batch tiles per core
LR = 0.01
F32 = mybir.dt.float32
MD = mybir.dt.bfloat16  # matmul dtype

_CACHE = {}


def _build_nc():
    nc = bacc.Bacc(
        trn_type="TRN2", target_bir_lowering=False, debug=False, num_devices=NCORES
    )
    obs = nc.declare_dram_parameter("obs", [BPC, OBS], F32, isOutput=False).ap()
    u0 = nc.declare_dram_parameter("u0", [BPC, 4], F32, isOutput=False).ap()
    w1 = nc.declare_dram_parameter("w1", [OBS, HID], F32, isOutput=False).ap()
    w2 = nc.declare_dram_parameter("w2", [HID, HID], F32, isOutput=False).ap()
    w3 = nc.declare_dram_parameter("w3", [HID, NQP], F32, isOutput=False).ap()
    b1 = nc.declare_dram_parameter("b1", [128, 4], F32, isOutput=False).ap()
    b2 = nc.declare_dram_parameter("b2", [128, 4], F32, isOutput=False).ap()
    b3 = nc.declare_dram_parameter("b3", [NQP, 1], F32, isOutput=False).ap()
    uo = nc.declare_dram_parameter("uo", [BPC, 4], F32, isOutput=True).ap()

    AF = mybir.ActivationFunctionType

    with tile.TileContext(nc) as tc:
        from contextlib import ExitStack

        with ExitStack() as ctx:
            singles = ctx.enter_context(tc.tile_pool(name="singles", bufs=1))
            p_obsf = ctx.enter_context(tc.tile_pool(name="obsf", bufs=2))
            p_obsb = ctx.enter_context(tc.tile_pool(name="obsb", bufs=2))
            p_obsT = ctx.enter_context(tc.tile_pool(name="obsT", bufs=2))
            p_y1 = ctx.enter_context(tc.tile_pool(name="y1", bufs=2))
            p_y2 = ctx.enter_context(tc.tile_pool(name="y2", bufs=2))
            p_qp = ctx.enter_context(tc.tile_pool(name="qp", bufs=2))
            p_cf = ctx.enter_context(tc.tile_pool(name="cf", bufs=1))
            pp_ot = ctx.enter_context(tc.tile_pool(name="ppot", bufs=2, space="PSUM"))
            pp_y1 = ctx.enter_context(tc.tile_pool(name="ppy1", bufs=2, space="PSUM"))
            pp_y2 = ctx.enter_context(tc.tile_pool(name="ppy2", bufs=2, space="PSUM"))
            pp_z3 = ctx.enter_context(tc.tile_pool(name="ppz3", bufs=1, space="PSUM"))
            pp_qpt = ctx.enter_context(tc.tile_pool(name="ppqpt", bufs=1, space="PSUM"))

            # ---- one-time: weights (cast to bf16), biases, identities ----
            w1f = singles.tile([128, 2, HID], F32)
            nc.sync.dma_start(out=w1f, in_=w1.rearrange("(kc p) m -> p kc m", p=128))
            w1s = singles.tile([128, 2, HID], MD)
            nc.vector.tensor_copy(out=w1s, in_=w1f)

            w2f = singles.tile([128, 4, HID], F32)
            nc.sync.dma_start(out=w2f, in_=w2.rearrange("(kc p) m -> p kc m", p=128))
            w2s = singles.tile([128, 4, HID], MD)
            nc.vector.tensor_copy(out=w2s, in_=w2f)

            w3f = singles.tile([128, 4, NQP], F32)
            nc.sync.dma_start(out=w3f, in_=w3.rearrange("(kc p) m -> p kc m", p=128))
            w3s = singles.tile([128, 4, NQP], MD)
            nc.vector.tensor_copy(out=w3s, in_=w3f)

            b1s = singles.tile([128, 4], F32)
            nc.sync.dma_start(out=b1s, in_=b1)
            b2s = singles.tile([128, 4], F32)
            nc.sync.dma_start(out=b2s, in_=b2)
            b3s = singles.tile([NQP, 1], F32)
            nc.sync.dma_start(out=b3s, in_=b3)

            ident = singles.tile([128, 128], MD)
            masks.make_identity(nc, ident[:])
            id8 = singles.tile([8, 8], F32)
            masks.make_identity(nc, id8[:])

            obs_t = obs.rearrange("(t c p) f -> t p c f", p=128, c=4)
            u0_t = u0.rearrange("(t c p) j -> p t c j", p=128, c=4)
            uo_t = uo.rearrange("(t c p) j -> p t c j", p=128, c=4)

            # whole-core u_init and the batch-major sigmoid(z3) accumulator
            u0all = singles.tile([128, NT, 4, 4], F32)
            nc.sync.dma_start(out=u0all, in_=u0_t)
            qpall = singles.tile([128, NT, 4, NQP], F32)

            for it in range(NT):
                # load + cast obs tile [128, 4, 256]
                obsf = p_obsf.tile([128, 4, OBS], F32)
                nc.sync.dma_start(out=obsf, in_=obs_t[it])
                obsb = p_obsb.tile([128, 4, OBS], MD)
                nc.vector.tensor_copy(out=obsb, in_=obsf)

                # transpose to obsT [256, BT] as 2 chunks of [128, BT]
                obsT = []
                for f in range(2):
                    ps = pp_ot.tile([128, BT], MD, tag="ot")
                    for c in range(4):
                        nc.tensor.transpose(
                            ps[:, c * 128 : (c + 1) * 128],
                            obsb[:, c, f * 128 : (f + 1) * 128],
                            ident[:],
                        )
                    ot = p_obsT.tile([128, BT], MD, tag=f"obsT{f}")
                    nc.vector.tensor_copy(out=ot, in_=ps)
                    obsT.append(ot)

                # layer 1: y1T[m] = relu(W1[:, m].T @ obsT + b1[m]) — drained on
                # DVE (fused bias-add + max(0)) to keep ACT free for layer 2
                y1 = []
                for m in range(4):
                    ps = pp_y1.tile([128, BT], F32, tag="y1")
                    for kc in range(2):
                        nc.tensor.matmul(
                            ps,
                            w1s[:, kc, m * 128 : (m + 1) * 128],
                            obsT[kc],
                            start=(kc == 0),
                            stop=(kc == 1),
                        )
                    t = p_y1.tile([128, BT], MD, tag=f"y1_{m}")
                    nc.vector.tensor_scalar(
                        t,
                        ps,
                        b1s[:, m : m + 1],
                        0.0,
                        mybir.AluOpType.add,
                        mybir.AluOpType.max,
                    )
                    y1.append(t)

                # layer 2
                y2 = []
                for m in range(4):
                    ps = pp_y2.tile([128, BT], F32, tag="y2")
                    for kc in range(4):
                        nc.tensor.matmul(
                            ps,
                            w2s[:, kc, m * 128 : (m + 1) * 128],
                            y1[kc],
                            start=(kc == 0),
                            stop=(kc == 3),
                        )
                    t = p_y2.tile([128, BT], MD, tag=f"y2_{m}")
                    nc.scalar.activation(
                        out=t, in_=ps, func=AF.Relu, bias=b2s[:, m : m + 1], scale=1.0
                    )
                    y2.append(t)

                # layer 3 (only the 8 useful output columns), sigmoid
                ps3 = pp_z3.tile([NQP, BT], F32, tag="z3")
                for kc in range(4):
                    nc.tensor.matmul(
                        ps3, w3s[:, kc, :], y2[kc], start=(kc == 0), stop=(kc == 3)
                    )
                qpT = p_qp.tile([NQP, BT], F32, tag="qpT")
                nc.scalar.activation(
                    out=qpT, in_=ps3, func=AF.Sigmoid, bias=b3s[:, 0:1], scale=1.0
                )

                # transpose to batch-major [128, 4 chunks, 8] and stash
                psq = pp_qpt.tile([128, 4, NQP], F32, tag="qpt")
                for c in range(4):
                    nc.tensor.transpose(
                        psq[:, c, :], qpT[:, c * 128 : (c + 1) * 128], id8[:]
                    )
                nc.vector.tensor_copy(out=qpall[:, it], in_=psq)

            # ---- closed-form 100-step update, batched over the whole core ----
            q = qpall[:, :, :, 0:4]
            p = qpall[:, :, :, 4:8]
            SH = [128, NT, 4, 4]
            TS, ALU = nc.vector.tensor_scalar, mybir.AluOpType

            a = p_cf.tile(SH, F32, tag="a")  # a = 1 - 2*lr*q
            TS(a, q, -2.0 * LR, 1.0, ALU.mult, ALU.add)
            a2 = p_cf.tile(SH, F32, tag="a2")
            nc.vector.tensor_mul(a2, a, a)
            a4 = p_cf.tile(SH, F32, tag="a4")
            nc.vector.tensor_mul(a4, a2, a2)
            a8 = p_cf.tile(SH, F32, tag="a8")
            nc.vector.tensor_mul(a8, a4, a4)
            a16 = p_cf.tile(SH, F32, tag="a16")
            nc.vector.tensor_mul(a16, a8, a8)
            a32 = p_cf.tile(SH, F32, tag="a32")
            nc.vector.tensor_mul(a32, a16, a16)
            a64 = p_cf.tile(SH, F32, tag="a64")
            nc.vector.tensor_mul(a64, a32, a32)
            a96 = p_cf.tile(SH, F32, tag="a96")
            nc.vector.tensor_mul(a96, a64, a32)
            A = p_cf.tile(SH, F32, tag="A")
            nc.vector.tensor_mul(A, a96, a4)

            n1 = p_cf.tile(SH, F32, tag="n1")  # 0.5*(1-A)
            TS(n1, A, -0.5, 0.5, ALU.mult, ALU.add)
            rq = p_cf.tile(SH, F32, tag="rq")
            nc.vector.reciprocal(rq, q)
            r = p_cf.tile(SH, F32, tag="r")
            nc.vector.tensor_mul(r, p, rq)
            tt = p_cf.tile(SH, F32, tag="tt")
            nc.vector.tensor_mul(tt, r, n1)
            mm = p_cf.tile(SH, F32, tag="mm")
            nc.vector.tensor_mul(mm, A, u0all)
            uob = p_cf.tile(SH, F32, tag="uob")
            nc.vector.tensor_sub(uob, mm, tt)
            nc.sync.dma_start(out=uo_t, in_=uob)
    nc.finalize()
    return nc


def _get_nc():
    if "nc" not in _CACHE:
        _CACHE["nc"] = _build_nc()
    return _CACHE["nc"]


def kernel(obs, x_init, u_init, W1, b1, W2, b2, W3, b3):
    obs = np.ascontiguousarray(np.asarray(obs, dtype=np.float32))
    u_init = np.ascontiguousarray(np.asarray(u_init, dtype=np.float32))
    W1 = np.asarray(W1, dtype=np.float32)
    W2 = np.asarray(W2, dtype=np.float32)
    W3 = np.asarray(W3, dtype=np.float32)
    b1 = np.asarray(b1, dtype=np.float32)
    b2 = np.asarray(b2, dtype=np.float32)
    b3 = np.asarray(b3, dtype=np.float32)

    # only columns 12:16 (q_u) and 28:32 (p_u) of the MLP head are used
    w3u = np.ascontiguousarray(np.concatenate([W3[:, 12:16], W3[:, 28:32]], axis=1))
    b3u = np.ascontiguousarray(
        np.concatenate([b3[12:16], b3[28:32]])[:, None]
    )
    b1p = np.ascontiguousarray(b1.reshape(4, 128).T)  # [128, m] chunks
    b2p = np.ascontiguousarray(b2.reshape(4, 128).T)
    w1c = np.ascontiguousarray(W1)
    w2c = np.ascontiguousarray(W2)

    nc = _get_nc()
    in_maps = []
    for i in range(NCORES):
        in_maps.append(
            {
                "obs": obs[i * BPC : (i + 1) * BPC],
                "u0": u_init[i * BPC : (i + 1) * BPC],
                "w1": w1c,
                "w2": w2c,
                "w3": w3u,
                "b1": b1p,
                "b2": b2p,
                "b3": b3u,
            }
        )
    import os

    kw = {}
    if os.environ.get("BASSK_TRACE"):
        kw = {"trace": True, "tmpdir": os.environ.get("BASSK_TRACE_DIR") or None}
    res = run_bass_kernel_spmd(nc, in_maps, list(range(NCORES)), **kw)
    _CACHE["last_result"] = res
    out = np.concatenate([res.results[i]["uo"] for i in range(NCORES)], axis=0)
    return out.astype(np.float32)
